# revision 22
# baseline (speedup 1.0000x reference)
"""DynamicEdgeConv graph classification on 8 Trainium2 NeuronCores (Bass/Tile).

Per core (nodes row-sharded 2048/core, params replicated):
  - Distance block R[m,j] = 2*S*x_m.x_j - S*sq_j on TensorE, plus a rank-1
    matmul adding B_HUGE = 3*2^32 so fp32 PSUM rounding quantizes S*R to a
    1024-step integer grid (q = round(S*R)).
  - One DVE scalar_tensor_tensor pass subtracts B_HUGE and adds a
    within-segment iota: packed values  q*1024 + (j % 1024)  carry their own
    column index through top-k selection.
  - vector.max (top-8) per 1024-wide segment + 3 rounds of max/match_replace
    give the top-24 packed candidates; indices decode with int shifts/masks.
  - Top-16 candidates are neighbors unconditionally; candidates 17-24 are
    re-ranked by exact distances recomputed from dma_gather'ed x_j rows;
    top-4 of the 8 are kept (flag-masked in the aggregation).
  - Edge MLP folded: h = lrelu(A[n] + P[j]), A = x@Wa + t, P = x@Wb
    (BatchNorm and the mean's 1/20 folded into weights host-side).
  - AllGather (transposed features + sq row, and row-major features) between
    layers; trailing MLP row-sharded with TensorE transposes.
"""

import numpy as np

N = 16384
NC = 8
RSH = N // NC            # 2048 rows per core
CH = RSH // 128          # 16 chunks
F1 = 116
E = 64
K = 20
NCAND = 24
NFIX = 16
NTAIL = NCAND - NFIX     # 8
SEG = 1024
NSEG = N // SEG
EPS = 1e-5
SLOPE = 0.2
BH = float(3 * 2**32)
SCALES = [41.0 * 1024, 105.4 * 1024, 273.9 * 1024]
NEG = -3.0e38
TXW1 = 128               # T_X row width layer 1 (116 padded to 128 floats)

_CACHE = {}


def _fold(W, g, b, m, v):
    s = (g / np.sqrt(v + EPS)).astype(np.float32)
    F = W.shape[0] // 2
    Wa = ((W[:F] - W[F:]) * s[None, :] / K).astype(np.float32)
    Wb = ((W[F:]) * s[None, :] / K).astype(np.float32)
    t = ((b - m * s) / K).astype(np.float32)
    return Wa, Wb, t


def _build(nc):
    import concourse.bass as bass
    import concourse.mybir as mybir
    import concourse.tile as tile

    dt = mybir.dt
    Alu = mybir.AluOpType
    Act = mybir.ActivationFunctionType
    f32, i32, i16, u32 = dt.float32, dt.int32, dt.int16, dt.uint32
    X = mybir.AxisListType.X

    din = {}

    def inp(name, shape, dtype=f32):
        din[name] = nc.dram_tensor(name, list(shape), dtype, kind="ExternalInput")
        return din[name]

    LA1 = inp("LA1", [F1 + 2, RSH])
    XR1 = inp("XR1", [F1 + 2, N])
    XROW1 = inp("XROW1", [128, CH * F1])
    A1 = inp("A1", [128, CH * E])
    TP1 = inp("TP1", [N, E])
    TX1 = inp("TX1", [N, TXW1])
    IOTA = inp("IOTA", [128, 2 * SEG])
    ONESN = inp("ONESN", [1, 128])
    BROWN = inp("BROWN", [1, N])
    PONE = inp("PONE", [1, RSH])
    IDENT = inp("IDENT", [128, 128])
    W2aug = inp("W2aug", [E + 1, E])
    W3aug = inp("W3aug", [E + 1, E])
    Wb2h = inp("Wb2h", [E, E])
    Wb3h = inp("Wb3h", [E, E])
    WC = inp("WC", [3 * E, 512])
    WM1 = inp("WM1", [512, 256])
    BM1 = inp("BM1", [1, 256])
    WM2 = inp("WM2", [256, 128])
    BM2 = inp("BM2", [1, 128])
    WCLF = inp("WCLF", [128, 2])
    BCLF = inp("BCLF", [1, 2])

    OUT_L = nc.dram_tensor("OUT_L", [RSH, 2], f32, kind="ExternalOutput")

    OUT_F = nc.dram_tensor("OUT_F", [RSH, 128], f32, kind="ExternalOutput")

    with tile.TileContext(nc) as tc:
        with tc.tile_pool(name="dram", bufs=1, space="DRAM") as dram, \
             tc.tile_pool(name="const", bufs=1) as cpool, \
             tc.tile_pool(name="pp", bufs=1) as pp:

            TP = dram.tile([N, E], f32, tag="TP")
            ag1_in = dram.tile([E + 1, RSH], f32, tag="ag1i")

            ag2_in = dram.tile([RSH, E], f32, tag="ag2i")

            DIDX = dram.tile([NCAND * RSH], i16, tag="didx")

            iota_sb = cpool.tile([128, 2 * SEG], f32, tag="iota")
            nc.sync.dma_start(iota_sb[:], IOTA.ap())
            onesn_sb = cpool.tile([1, 128], f32, tag="onesn")
            nc.sync.dma_start(onesn_sb[:], ONESN.ap())
            ident_sb = cpool.tile([128, 128], f32, tag="ident")
            nc.sync.dma_start(ident_sb[:], IDENT.ap())
            ws = {}
            for nm, t, sh in [("W2aug", W2aug, [E + 1, E]), ("W3aug", W3aug, [E + 1, E]),
                              ("Wb2h", Wb2h, [E, E]), ("Wb3h", Wb3h, [E, E]),
                              ("BM1", BM1, [1, 256]), ("BM2", BM2, [1, 128]),
                              ("WCLF", WCLF, [128, 2]), ("BCLF", BCLF, [1, 2])]:
                ws[nm] = cpool.tile(sh, f32, tag=nm, name=nm)
                nc.sync.dma_start(ws[nm][:], t.ap())

            # persistent (tag-shared slots reused across phases)
            xT_loc = [pp.tile([E, RSH], f32, tag=f"xT{l}", name=f"xT{l}") for l in range(3)]
            acc_l = [pp.tile([128, CH * E], f32, tag=f"acc{l}", name=f"acc{l}") for l in range(3)]

            # ---------------- selection + edges for one layer ----------------
            def do_layer(li, XRa, lhsTa, Fin, xrow3d, A_flat, A3d, TXa, txw, TPa):
                with tc.tile_pool(name=f"sel{li}", bufs=3) as sp, \
                     tc.tile_pool(name=f"selc{li}", bufs=1) as scp, \
                     tc.tile_pool(name=f"psumD{li}", bufs=2, space="PSUM") as pps:
                    idx_all = scp.tile([128, CH * NCAND], i32, tag="idxall")
                    for b in range(CH):
                        Ctab = sp.tile([128, NSEG * 8], f32, tag="Ctab")
                        for g in range(NSEG // 2):
                            ps = pps.tile([128, 2 * SEG], f32, tag="dps")
                            for h in range(2 * SEG // 512):
                                sl = ps[:, h * 512:(h + 1) * 512]
                                nc.tensor.matmul(
                                    sl, lhsTa[:, b * 128:(b + 1) * 128],
                                    XRa[:, g * 2 * SEG + h * 512:g * 2 * SEG + (h + 1) * 512],
                                    start=True, stop=True)
                            pk = sp.tile([128, 2 * SEG], f32, tag="pk")
                            nc.vector.scalar_tensor_tensor(
                                pk[:], ps[:], -BH, iota_sb[:],
                                op0=Alu.add, op1=Alu.add)
                            nc.vector.max(Ctab[:, g * 16:g * 16 + 8], pk[:, 0:SEG])
                            nc.vector.max(Ctab[:, g * 16 + 8:g * 16 + 16],
                                          pk[:, SEG:2 * SEG])
                        W24 = sp.tile([128, NCAND], f32, tag="W24")
                        cw = sp.tile([128, NSEG * 8], f32, tag="cw")
                        cw2 = sp.tile([128, NSEG * 8], f32, tag="cw2")
                        nc.vector.tensor_copy(cw[:], Ctab[:])
                        nc.vector.max(W24[:, 0:8], cw[:])
                        nc.vector.match_replace(cw2[:], W24[:, 0:8], cw[:], NEG)
                        nc.vector.max(W24[:, 8:16], cw2[:])
                        nc.vector.match_replace(cw[:], W24[:, 8:16], cw2[:], NEG)
                        nc.vector.max(W24[:, 16:24], cw[:])
                        pos = sp.tile([128, NCAND], u32, tag="pos")
                        for rr in range(3):
                            nc.vector.max_index(pos[:, rr * 8:(rr + 1) * 8],
                                                W24[:, rr * 8:(rr + 1) * 8], Ctab[:])
                        wi = sp.tile([128, NCAND], i32, tag="wi")
                        nc.vector.tensor_copy(wi[:], W24[:])
                        jlo = sp.tile([128, NCAND], i32, tag="jlo")
                        nc.vector.tensor_scalar(jlo[:], wi[:], 1023, None,
                                                op0=Alu.bitwise_and)
                        seg_t = sp.tile([128, NCAND], i32, tag="seg")
                        nc.vector.tensor_scalar(seg_t[:], pos[:].bitcast(i32), 3, 10,
                                                op0=Alu.arith_shift_right,
                                                op1=Alu.logical_shift_left)
                        nc.vector.tensor_tensor(
                            idx_all[:].rearrange("p (k c) -> p k c", c=CH)[:, :, b],
                            seg_t[:], jlo[:], op=Alu.add)

                    idx16 = scp.tile([128, CH * NCAND], i16, tag="idx16")
                    nc.vector.tensor_copy(idx16[:], idx_all[:])
                    nc.sync.dma_start(
                        DIDX[:].rearrange("(k b p) -> p k b", p=128, b=CH),
                        idx16[:].rearrange("p (k c) -> p k c", c=CH))

                with tc.tile_pool(name=f"edge{li}", bufs=2) as ep, \
                     tc.tile_pool(name=f"edgec{li}", bufs=1) as ecp:
                    nl = NCAND * RSH
                    elist = ecp.tile([128, nl // 16], i16, tag="elist")
                    for r in range(8):
                        nc.sync.dma_start(
                            elist[r * 16:(r + 1) * 16, :],
                            DIDX[:].rearrange("(s p) -> p s", p=16))

                    acc = acc_l[li]
                    nc.vector.memset(acc[:], 0.0)

                    # tail refine: exact r = 2*x_m.x_j - sq_j for slots 16..23
                    r_all = ecp.tile([128, CH * NTAIL], f32, tag="rall")
                    rv = r_all[:].rearrange("p (c t) -> p c t", t=NTAIL)
                    for kk in range(NFIX, NCAND):
                        gx = ep.tile([128, CH * txw], f32, tag="gx")
                        gxv = gx[:].rearrange("p (c e) -> p c e", e=txw)
                        nc.gpsimd.dma_gather(
                            gxv, TXa, elist[:, kk * 128:(kk + 1) * 128],
                            RSH, RSH, txw, single_packet=False)
                        prod = ep.tile([128, CH * Fin], f32, tag="scr")
                        pv = prod[:].rearrange("p (c f) -> p c f", f=Fin)
                        nc.vector.tensor_tensor(pv, gxv[:, :, 0:Fin], xrow3d,
                                                op=Alu.mult)
                        dot = ep.tile([128, CH, 1], f32, tag="dot")
                        nc.vector.reduce_sum(dot[:], pv, axis=X)
                        sqg = ep.tile([128, CH * Fin], f32, tag="scr")
                        sgv = sqg[:].rearrange("p (c f) -> p c f", f=Fin)
                        nc.scalar.square(sgv, gxv[:, :, 0:Fin])
                        sqv = ep.tile([128, CH, 1], f32, tag="sqv")
                        nc.vector.reduce_sum(sqv[:], sgv, axis=X)
                        nc.vector.scalar_tensor_tensor(
                            rv[:, :, kk - NFIX:kk - NFIX + 1],
                            dot[:], 2.0, sqv[:],
                            op0=Alu.mult, op1=Alu.subtract)
                    srt = ecp.tile([128, CH * NTAIL], f32, tag="srt")
                    sv = srt[:].rearrange("p (c t) -> p c t", t=NTAIL)
                    for b in range(CH):
                        nc.vector.max(sv[:, b, :], rv[:, b, :])
                    flags = ecp.tile([128, CH * NTAIL], f32, tag="flags")
                    fv = flags[:].rearrange("p (c t) -> p c t", t=NTAIL)
                    thr_b, rv_b = bass.broadcast_tensor_aps(sv[:, :, 3:4], rv)
                    nc.vector.tensor_tensor(fv, rv_b, thr_b, op=Alu.is_ge)

                    # aggregate all 24 slots (tail slots flag-masked)
                    for kk in range(NCAND):
                        gp = ep.tile([128, CH * E], f32, tag="gp")
                        nc.gpsimd.dma_gather(
                            gp[:].rearrange("p (c e) -> p c e", e=E),
                            TPa, elist[:, kk * 128:(kk + 1) * 128],
                            RSH, RSH, E, single_packet=False)
                        tmp = ep.tile([128, CH * E], f32, tag="tmp")
                        nc.vector.tensor_add(tmp[:], gp[:], A_flat)
                        nc.vector.scalar_tensor_tensor(
                            tmp[:], tmp[:], SLOPE, tmp[:],
                            op0=Alu.mult, op1=Alu.max)
                        if kk >= NFIX:
                            tv = tmp[:].rearrange("p (c e) -> p c e", e=E)
                            fl_b, tv_b = bass.broadcast_tensor_aps(
                                fv[:, :, kk - NFIX:kk - NFIX + 1], tv)
                            nc.vector.tensor_tensor(tv_b, tv_b, fl_b, op=Alu.mult)
                        nc.vector.tensor_add(acc[:], acc[:], tmp[:])
                return acc

            # ---------------- inter-layer ----------------
            def interlayer(li, acc, Wbh, Waug):
                Snext = SCALES[li + 1]
                ag1_out = dram.tile([NC * (E + 1), RSH], f32, tag=f"ag1o{li}",
                                    name=f"ag1o{li}", addr_space="Shared")
                ag2_out = dram.tile([N, E], f32, tag=f"ag2o{li}",
                                    name=f"ag2o{li}", addr_space="Shared")
                with tc.tile_pool(name=f"il{li}", bufs=2) as ip, \
                     tc.tile_pool(name=f"psumT{li}", bufs=2, space="PSUM") as pt:
                    accv = acc[:].rearrange("p (c e) -> p c e", e=E)
                    sq2 = ip.tile([128, CH * E], f32, tag="sq2")
                    nc.scalar.square(sq2[:], acc[:])
                    sqr = ip.tile([128, CH, 1], f32, tag="sqr")
                    nc.vector.reduce_sum(
                        sqr[:], sq2[:].rearrange("p (c e) -> p c e", e=E), axis=X)
                    nc.vector.tensor_scalar(sqr[:], sqr[:], float(Snext), None,
                                            op0=Alu.mult)
                    xt = xT_loc[li + 1]
                    for b in range(CH):
                        tp = pt.tile([E, 128], f32, tag="tp")
                        nc.tensor.transpose(tp[:], accv[:, b, :], ident_sb[:])
                        nc.scalar.copy(xt[:, b * 128:(b + 1) * 128], tp[:])
                    nc.sync.dma_start(ag1_in[0:E, :], xt[:])
                    nc.sync.dma_start(
                        ag1_in[E:E + 1, :].rearrange("o (b p) -> p b o", p=128),
                        sqr[:])
                    nc.sync.dma_start(
                        ag2_in[:].rearrange("(b p) e -> p b e", p=128), accv)
                    nc.gpsimd.collective_compute(
                        "AllGather", Alu.bypass, ins=[ag1_in.opt()],
                        outs=[ag1_out.opt()], replica_groups=[list(range(NC))])
                    nc.gpsimd.collective_compute(
                        "AllGather", Alu.bypass, ins=[ag2_in.opt()],
                        outs=[ag2_out.opt()], replica_groups=[list(range(NC))])

                    xr = pp.tile([E + 2, N], f32, tag="xrbig")
                    for r in range(NC):
                        nc.sync.dma_start(xr[0:E, r * RSH:(r + 1) * RSH],
                                          ag1_out[r * (E + 1):r * (E + 1) + E, :])
                        nc.sync.dma_start(xr[E:E + 1, r * RSH:(r + 1) * RSH],
                                          ag1_out[r * (E + 1) + E:(r + 1) * (E + 1), :])
                    nc.vector.tensor_scalar(xr[0:E, :], xr[0:E, :],
                                            float(2.0 * Snext), None, op0=Alu.mult)
                    nc.sync.dma_start(xr[E + 1:E + 2, :], BROWN.ap())
                    lh = pp.tile([E + 2, RSH], f32, tag="lhsT")
                    nc.vector.tensor_copy(lh[0:E, :], xt[:])
                    nc.vector.memset(lh[E:E + 1, :], -1.0)
                    nc.sync.dma_start(lh[E + 1:E + 2, :], PONE.ap())
                    a2 = pp.tile([128, CH * E], f32, tag="Atile")
                    psb = pp.tile([128, (N // 4 // 128) * E], f32, tag="psb")
                    with tc.tile_pool(name=f"psumA{li}", bufs=2, space="PSUM") as pa:
                        for b in range(CH):
                            aps = pa.tile([128, E], f32, tag="aps")
                            nc.tensor.matmul(aps[:], lh[0:E + 1, b * 128:(b + 1) * 128],
                                             Waug[:], start=True, stop=True)
                            nc.scalar.copy(a2[:, b * E:(b + 1) * E], aps[:])
                        for half in range(4):
                            for jb in range(N // 4 // 128):
                                jg = half * (N // 4 // 128) + jb
                                pps2 = pa.tile([128, E], f32, tag="pps2")
                                nc.tensor.matmul(
                                    pps2[:], xr[0:E, jg * 128:(jg + 1) * 128],
                                    Wbh[:], start=True, stop=True)
                                nc.scalar.copy(psb[:, jb * E:(jb + 1) * E], pps2[:])
                            nc.sync.dma_start(
                                TP[half * (N // 4):(half + 1) * (N // 4), :]
                                .rearrange("(jb p) e -> p jb e", p=128),
                                psb[:].rearrange("p (jb e) -> p jb e", e=E))
                    return xr, lh, a2, ag2_out

            # ---------------- layer 1 ----------------
            xr1 = pp.tile([F1 + 2, N], f32, tag="xrbig")
            nc.sync.dma_start(xr1[:], XR1.ap())
            la1 = pp.tile([F1 + 2, RSH], f32, tag="lhsT")
            nc.sync.dma_start(la1[:], LA1.ap())
            xrow1 = pp.tile([128, CH * F1], f32, tag="xrow1")
            nc.sync.dma_start(xrow1[:], XROW1.ap())
            a1 = pp.tile([128, CH * E], f32, tag="Atile")
            nc.sync.dma_start(a1[:], A1.ap())

            do_layer(0, xr1[:], la1[:], F1,
                     xrow1[:].rearrange("p (c f) -> p c f", f=F1),
                     a1[:], a1[:].rearrange("p (c e) -> p c e", e=E),
                     TX1.ap(), TXW1, TP1.ap())

            xr2, lh2, a2, ag2o_0 = interlayer(0, acc_l[0], ws["Wb2h"], ws["W2aug"])
            do_layer(1, xr2[:], lh2[:], E,
                     acc_l[0][:].rearrange("p (c f) -> p c f", f=E),
                     a2[:], a2[:].rearrange("p (c e) -> p c e", e=E),
                     ag2o_0[:], E, TP[:])

            xr3, lh3, a3, ag2o_1 = interlayer(1, acc_l[1], ws["Wb3h"], ws["W3aug"])
            do_layer(2, xr3[:], lh3[:], E,
                     acc_l[1][:].rearrange("p (c f) -> p c f", f=E),
                     a3[:], a3[:].rearrange("p (c e) -> p c e", e=E),
                     ag2o_1[:], E, TP[:])

            # x3^T for trailing
            with tc.tile_pool(name="psumT3", bufs=2, space="PSUM") as pt3:
                accv = acc_l[2][:].rearrange("p (c e) -> p c e", e=E)
                for b in range(CH):
                    tp = pt3.tile([E, 128], f32, tag="tp3")
                    nc.tensor.transpose(tp[:], accv[:, b, :], ident_sb[:])
                    nc.scalar.copy(xT_loc[0][:, b * 128:(b + 1) * 128], tp[:])

            # ---------------- trailing MLP ----------------
            with tc.tile_pool(name="trail", bufs=2) as tr, \
                 tc.tile_pool(name="psumM", bufs=2, space="PSUM") as pm:
                wc_sb = pp.tile([E, 3 * 512], f32, tag="Atile")
                nc.sync.dma_start(
                    wc_sb[:].rearrange("f (l o) -> f l o", o=512),
                    WC.ap().rearrange("(l f) o -> f l o", f=E))
                wm1_sb = pp.tile([128, 4 * 256], f32, tag="lhsT")
                nc.sync.dma_start(
                    wm1_sb[:].rearrange("p (q o) -> p q o", o=256),
                    WM1.ap().rearrange("(q p) o -> p q o", p=128))
                wm2_sb = tr.tile([128, 2 * 128], f32, tag="wm2", bufs=1)
                nc.sync.dma_start(
                    wm2_sb[:].rearrange("p (q o) -> p q o", o=128),
                    WM2.ap().rearrange("(q p) o -> p q o", p=128))
                h1T = [pp.tile([128, RSH], f32, tag="xrbig", name="h1T0")] + [
                    tr.tile([128, RSH], f32, tag=tg, name=f"h1T_{tg}", bufs=1)
                    for tg in ("h1T1", "h1T2", "h1T3")]
                h2T = [tr.tile([128, RSH], f32, tag=tg, name=tg, bufs=1)
                       for tg in ("h2T0", "h2T1")]
                featT = pp.tile([128, RSH], f32, tag="xrow1")
                feat_sb = pp.tile([128, CH * 128], f32, tag="psb")

                xts = [xT_loc[1], xT_loc[2], xT_loc[0]]
                wm1v = wm1_sb[:].rearrange("p (q o) -> p q o", o=256)
                wm2v = wm2_sb[:].rearrange("p (q o) -> p q o", o=128)
                for b in range(CH):
                    hps = pm.tile([128, 512], f32, tag="mm")
                    for l3 in range(3):
                        nc.tensor.matmul(hps[:], xts[l3][:, b * 128:(b + 1) * 128],
                                         wc_sb[:, l3 * 512:(l3 + 1) * 512],
                                         start=(l3 == 0), stop=(l3 == 2))
                    hsb = tr.tile([128, 512], f32, tag="hsb")
                    nc.scalar.copy(hsb[:], hps[:])
                    for q in range(4):
                        tp = pm.tile([128, 128], f32, tag="tpq")
                        nc.tensor.transpose(tp[:], hsb[:, q * 128:(q + 1) * 128],
                                            ident_sb[:])
                        nc.scalar.copy(h1T[q][:, b * 128:(b + 1) * 128], tp[:])
                for b in range(CH):
                    h2ps = pm.tile([128, 256], f32, tag="mm")
                    for q in range(4):
                        nc.tensor.matmul(h2ps[:], h1T[q][:, b * 128:(b + 1) * 128],
                                         wm1v[:, q, :], start=(q == 0), stop=False)
                    nc.tensor.matmul(h2ps[:], onesn_sb[:], ws["BM1"][:],
                                     start=False, stop=True)
                    h2sb = tr.tile([128, 256], f32, tag="h2sb")
                    nc.scalar.copy(h2sb[:], h2ps[:])
                    nc.vector.scalar_tensor_tensor(
                        h2sb[:], h2sb[:], SLOPE, h2sb[:],
                        op0=Alu.mult, op1=Alu.max)
                    for q in range(2):
                        tp = pm.tile([128, 128], f32, tag="tpq")
                        nc.tensor.transpose(tp[:], h2sb[:, q * 128:(q + 1) * 128],
                                            ident_sb[:])
                        nc.scalar.copy(h2T[q][:, b * 128:(b + 1) * 128], tp[:])
                for b in range(CH):
                    fps = pm.tile([128, 128], f32, tag="mm")
                    for q in range(2):
                        nc.tensor.matmul(fps[:], h2T[q][:, b * 128:(b + 1) * 128],
                                         wm2v[:, q, :], start=(q == 0), stop=False)
                    nc.tensor.matmul(fps[:], onesn_sb[:], ws["BM2"][:],
                                     start=False, stop=True)
                    nc.scalar.copy(feat_sb[:, b * 128:(b + 1) * 128], fps[:])
                    nc.vector.scalar_tensor_tensor(
                        feat_sb[:, b * 128:(b + 1) * 128],
                        feat_sb[:, b * 128:(b + 1) * 128], SLOPE,
                        feat_sb[:, b * 128:(b + 1) * 128],
                        op0=Alu.mult, op1=Alu.max)
                    tp = pm.tile([128, 128], f32, tag="tpq")
                    nc.tensor.transpose(tp[:], feat_sb[:, b * 128:(b + 1) * 128],
                                        ident_sb[:])
                    nc.scalar.copy(featT[:, b * 128:(b + 1) * 128], tp[:])
                nc.sync.dma_start(
                    OUT_F.ap().rearrange("(b p) f -> p b f", p=128),
                    feat_sb[:].rearrange("p (b f) -> p b f", f=128))
                for b in range(CH):
                    lps = pm.tile([128, 2], f32, tag="mm")
                    nc.tensor.matmul(lps[:], featT[:, b * 128:(b + 1) * 128],
                                     ws["WCLF"][:], start=True, stop=False)
                    nc.tensor.matmul(lps[:], onesn_sb[:], ws["BCLF"][:],
                                     start=False, stop=True)
                    lsb = tr.tile([128, 2], f32, tag="lsb")
                    nc.scalar.copy(lsb[:], lps[:])
                    nc.sync.dma_start(OUT_L.ap()[b * 128:(b + 1) * 128, :], lsb[:])

    nc.compile()
    return nc


def _host_prep(inputs):
    x = np.ascontiguousarray(inputs["x"], dtype=np.float32)
    sq = (x * x).sum(1).astype(np.float32)
    Wa1, Wb1, t1 = _fold(inputs["W1"], inputs["g1"], inputs["b1"], inputs["m1"], inputs["v1"])
    Wa2, Wb2, t2 = _fold(inputs["W2"], inputs["g2"], inputs["b2"], inputs["m2"], inputs["v2"])
    Wa3, Wb3, t3 = _fold(inputs["W3"], inputs["g3"], inputs["b3"], inputs["m3"], inputs["v3"])
    S1, S2, S3 = SCALES
    P1 = (x @ Wb1).astype(np.float32)
    TX1 = np.zeros((N, TXW1), np.float32)
    TX1[:, :F1] = x

    common = {
        "XR1": np.concatenate([2.0 * S1 * x.T, S1 * sq[None, :],
                               np.full((1, N), BH, np.float32)], 0).astype(np.float32),
        "TP1": P1, "TX1": TX1,
        "IOTA": np.broadcast_to(np.arange(2 * SEG, dtype=np.float32) % SEG, (128, 2 * SEG)).copy(),
        "ONESN": -np.ones((1, 128), np.float32),
        "BROWN": np.full((1, N), BH, np.float32),
        "PONE": np.ones((1, RSH), np.float32),
        "IDENT": np.eye(128, dtype=np.float32),
        "W2aug": np.concatenate([Wa2, -t2[None, :]], 0),
        "W3aug": np.concatenate([Wa3, -t3[None, :]], 0),
        "Wb2h": (Wb2 / (2.0 * S2)).astype(np.float32),
        "Wb3h": (Wb3 / (2.0 * S3)).astype(np.float32),
        "WC": np.ascontiguousarray(inputs["Wc"], np.float32),
    }
    s4 = (inputs["g4"] / np.sqrt(inputs["v4"] + EPS)).astype(np.float32)
    t4 = (inputs["b4"] - inputs["m4"] * s4).astype(np.float32)
    common["WM1"] = (inputs["Wm1"] * s4[None, :]).astype(np.float32)
    common["BM1"] = -(inputs["bm1"] * s4 + t4)[None, :].astype(np.float32)
    s5 = (inputs["g5"] / np.sqrt(inputs["v5"] + EPS)).astype(np.float32)
    t5 = (inputs["b5"] - inputs["m5"] * s5).astype(np.float32)
    common["WM2"] = (inputs["Wm2"] * s5[None, :]).astype(np.float32)
    common["BM2"] = -(inputs["bm2"] * s5 + t5)[None, :].astype(np.float32)
    common["WCLF"] = np.ascontiguousarray(inputs["Wclf"], np.float32)
    common["BCLF"] = -np.ascontiguousarray(inputs["bclf"], np.float32)[None, :]

    in_maps = []
    for c in range(NC):
        xs = x[c * RSH:(c + 1) * RSH]
        m = dict(common)
        m["LA1"] = np.concatenate([xs.T, -np.ones((1, RSH), np.float32),
                                   np.ones((1, RSH), np.float32)], 0).astype(np.float32)
        m["XROW1"] = np.ascontiguousarray(
            xs.reshape(CH, 128, F1).transpose(1, 0, 2).reshape(128, CH * F1))
        A1v = (xs @ Wa1 + t1[None, :]).astype(np.float32)
        m["A1"] = np.ascontiguousarray(
            A1v.reshape(CH, 128, E).transpose(1, 0, 2).reshape(128, CH * E))
        in_maps.append(m)
    return in_maps


def kernel(**inputs):
    import concourse.bacc as bacc
    from concourse.bass_utils import run_bass_kernel_spmd

    if "nc" not in _CACHE:
        ncb = bacc.Bacc("TRN2", target_bir_lowering=False, debug=False,
                        num_devices=NC)
        _CACHE["nc"] = _build(ncb)
    ncb = _CACHE["nc"]
    in_maps = _host_prep(inputs)
    res = run_bass_kernel_spmd(ncb, in_maps, core_ids=list(range(NC)))
    logits = np.concatenate([res.results[c]["OUT_L"] for c in range(NC)], 0)
    feat = np.concatenate([res.results[c]["OUT_F"] for c in range(NC)], 0)
    return logits, feat


# revision 25
# speedup vs baseline: 1.4751x; 1.4751x over previous
"""DynamicEdgeConv graph classification on 8 Trainium2 NeuronCores (Bass/Tile).

Per core (nodes row-sharded 2048/core, params replicated):
  - Distance block R[m,j] = 2*S*x_m.x_j - S*sq_j on TensorE, plus a rank-1
    matmul adding B_HUGE = 3*2^32 so fp32 PSUM rounding quantizes S*R to a
    1024-step integer grid (q = round(S*R)).
  - One DVE scalar_tensor_tensor pass subtracts B_HUGE and adds a
    within-segment iota: packed values  q*1024 + (j % 1024)  carry their own
    column index through top-k selection.
  - vector.max (top-8) per 1024-wide segment + 3 rounds of max/match_replace
    give the top-24 packed candidates; indices decode with int shifts/masks.
  - Top-16 candidates are neighbors unconditionally; candidates 17-24 are
    re-ranked by exact distances recomputed from dma_gather'ed x_j rows;
    top-4 of the 8 are kept (flag-masked in the aggregation).
  - Edge MLP folded: h = lrelu(A[n] + P[j]), A = x@Wa + t, P = x@Wb
    (BatchNorm and the mean's 1/20 folded into weights host-side).
  - AllGather (transposed features + sq row, and row-major features) between
    layers; trailing MLP row-sharded with TensorE transposes.
"""

import numpy as np

N = 16384
NC = 8
RSH = N // NC            # 2048 rows per core
CH = RSH // 128          # 16 chunks
F1 = 116
E = 64
K = 20
NCAND = 24
NFIX = 16
NTAIL = NCAND - NFIX     # 8
SEG = 1024
NSEG = N // SEG
EPS = 1e-5
SLOPE = 0.2
BH = float(3 * 2**32)
SCALES = [41.0 * 1024, 105.4 * 1024, 273.9 * 1024]
NEG = -3.0e38
TXW1 = 128               # T_X row width layer 1 (116 padded to 128 floats)

_CACHE = {}


def _fold(W, g, b, m, v):
    s = (g / np.sqrt(v + EPS)).astype(np.float32)
    F = W.shape[0] // 2
    Wa = ((W[:F] - W[F:]) * s[None, :] / K).astype(np.float32)
    Wb = ((W[F:]) * s[None, :] / K).astype(np.float32)
    t = ((b - m * s) / K).astype(np.float32)
    return Wa, Wb, t


def _build(nc):
    import concourse.bass as bass
    import concourse.mybir as mybir
    import concourse.tile as tile

    dt = mybir.dt
    Alu = mybir.AluOpType
    Act = mybir.ActivationFunctionType
    f32, i32, i16, u32 = dt.float32, dt.int32, dt.int16, dt.uint32
    X = mybir.AxisListType.X

    din = {}

    def inp(name, shape, dtype=f32):
        din[name] = nc.dram_tensor(name, list(shape), dtype, kind="ExternalInput")
        return din[name]

    LA1 = inp("LA1", [F1 + 2, RSH])
    XR1 = inp("XR1", [F1 + 2, N])
    XROW1 = inp("XROW1", [128, CH * F1])
    A1 = inp("A1", [128, CH * E])
    TP1 = inp("TP1", [N, E])
    TX1 = inp("TX1", [N, TXW1])
    IOTA = inp("IOTA", [128, 2 * SEG])
    ONESN = inp("ONESN", [1, 128])
    BROWN = inp("BROWN", [1, N])
    PONE = inp("PONE", [1, RSH])
    IDENT = inp("IDENT", [128, 128])
    W2aug = inp("W2aug", [E + 1, E])
    W3aug = inp("W3aug", [E + 1, E])
    Wb2h = inp("Wb2h", [E, E])
    Wb3h = inp("Wb3h", [E, E])
    WC = inp("WC", [3 * E, 512])
    WM1 = inp("WM1", [512, 256])
    BM1 = inp("BM1", [1, 256])
    WM2 = inp("WM2", [256, 128])
    BM2 = inp("BM2", [1, 128])
    WCLF = inp("WCLF", [128, 2])
    BCLF = inp("BCLF", [1, 2])

    OUT_L = nc.dram_tensor("OUT_L", [RSH, 2], f32, kind="ExternalOutput")

    OUT_F = nc.dram_tensor("OUT_F", [RSH, 128], f32, kind="ExternalOutput")

    with tile.TileContext(nc) as tc:
        with tc.tile_pool(name="dram", bufs=1, space="DRAM") as dram, \
             tc.tile_pool(name="const", bufs=1) as cpool, \
             tc.tile_pool(name="pp", bufs=1) as pp:

            TP = dram.tile([N, E], f32, tag="TP")
            ag1_in = dram.tile([E + 1, RSH], f32, tag="ag1i")

            ag2_in = dram.tile([RSH, E], f32, tag="ag2i")

            DIDX = dram.tile([NCAND * RSH], i16, tag="didx")

            iota_sb = cpool.tile([128, 2 * SEG], f32, tag="iota")
            nc.sync.dma_start(iota_sb[:], IOTA.ap())
            onesn_sb = cpool.tile([1, 128], f32, tag="onesn")
            nc.sync.dma_start(onesn_sb[:], ONESN.ap())
            ident_sb = cpool.tile([128, 128], f32, tag="ident")
            nc.sync.dma_start(ident_sb[:], IDENT.ap())
            ws = {}
            for nm, t, sh in [("W2aug", W2aug, [E + 1, E]), ("W3aug", W3aug, [E + 1, E]),
                              ("Wb2h", Wb2h, [E, E]), ("Wb3h", Wb3h, [E, E]),
                              ("BM1", BM1, [1, 256]), ("BM2", BM2, [1, 128]),
                              ("WCLF", WCLF, [128, 2]), ("BCLF", BCLF, [1, 2])]:
                ws[nm] = cpool.tile(sh, f32, tag=nm, name=nm)
                nc.sync.dma_start(ws[nm][:], t.ap())

            # persistent (tag-shared slots reused across phases)
            xT_loc = [pp.tile([E, RSH], f32, tag=f"xT{l}", name=f"xT{l}") for l in range(3)]
            acc_l = [pp.tile([128, CH * E], f32, tag=f"acc{l}", name=f"acc{l}") for l in range(3)]

            # ---------------- selection + edges for one layer ----------------
            def do_layer(li, XRa, lhsTa, Fin, xrow3d, A_flat, A3d, TXa, txw, TPa):
                with tc.tile_pool(name=f"sel{li}", bufs=3) as sp, \
                     tc.tile_pool(name=f"selc{li}", bufs=1) as scp, \
                     tc.tile_pool(name=f"psumD{li}", bufs=2, space="PSUM") as pps:
                    idx_all = scp.tile([128, CH * NCAND], i32, tag="idxall")
                    W24a = scp.tile([128, CH * NCAND], f32, tag="W24a")
                    posa = scp.tile([128, CH * NCAND], u32, tag="posa")
                    for b in range(CH):
                        Ctab = sp.tile([128, NSEG * 8], f32, tag="Ctab")
                        for g in range(NSEG // 2):
                            ps = pps.tile([128, 2 * SEG], f32, tag="dps")
                            for h in range(2 * SEG // 512):
                                sl = ps[:, h * 512:(h + 1) * 512]
                                nc.tensor.matmul(
                                    sl, lhsTa[:, b * 128:(b + 1) * 128],
                                    XRa[:, g * 2 * SEG + h * 512:g * 2 * SEG + (h + 1) * 512],
                                    start=True, stop=True)
                            pk = sp.tile([128, 2 * SEG], f32, tag="pk")
                            nc.vector.scalar_tensor_tensor(
                                pk[:], ps[:], -BH, iota_sb[:],
                                op0=Alu.add, op1=Alu.add)
                            nc.vector.max(Ctab[:, g * 16:g * 16 + 8], pk[:, 0:SEG])
                            nc.vector.max(Ctab[:, g * 16 + 8:g * 16 + 16],
                                          pk[:, SEG:2 * SEG])
                        W24 = W24a[:, b * NCAND:(b + 1) * NCAND]
                        cw = sp.tile([128, NSEG * 8], f32, tag="cw")
                        cw2 = sp.tile([128, NSEG * 8], f32, tag="cw2")
                        nc.vector.tensor_copy(cw[:], Ctab[:])
                        nc.vector.max(W24[:, 0:8], cw[:])
                        nc.vector.match_replace(cw2[:], W24[:, 0:8], cw[:], NEG)
                        nc.vector.max(W24[:, 8:16], cw2[:])
                        nc.vector.match_replace(cw[:], W24[:, 8:16], cw2[:], NEG)
                        nc.vector.max(W24[:, 16:24], cw[:])
                        pos = posa[:, b * NCAND:(b + 1) * NCAND]
                        for rr in range(3):
                            nc.vector.max_index(pos[:, rr * 8:(rr + 1) * 8],
                                                W24[:, rr * 8:(rr + 1) * 8], Ctab[:])
                    wi = scp.tile([128, CH * NCAND], i32, tag="wi")
                    nc.vector.tensor_copy(wi[:], W24a[:])
                    jlo = scp.tile([128, CH * NCAND], i32, tag="jlo")
                    nc.vector.tensor_scalar(jlo[:], wi[:], 1023, None,
                                            op0=Alu.bitwise_and)
                    seg_t = scp.tile([128, CH * NCAND], i32, tag="seg")
                    nc.vector.tensor_scalar(seg_t[:], posa[:].bitcast(i32), 3, 10,
                                            op0=Alu.arith_shift_right,
                                            op1=Alu.logical_shift_left)
                    nc.vector.tensor_tensor(idx_all[:], seg_t[:], jlo[:], op=Alu.add)

                    idx16 = scp.tile([128, CH * NCAND], i16, tag="idx16")
                    nc.vector.tensor_copy(
                        idx16[:].rearrange("p (k c) -> p k c", c=CH),
                        idx_all[:].rearrange("p (c k) -> p c k", k=NCAND)
                        .rearrange("p c k -> p k c"))
                    nc.sync.dma_start(
                        DIDX[:].rearrange("(k b p) -> p k b", p=128, b=CH),
                        idx16[:].rearrange("p (k c) -> p k c", c=CH))

                with tc.tile_pool(name=f"edge{li}", bufs=2) as ep, \
                     tc.tile_pool(name=f"edgec{li}", bufs=1) as ecp:
                    nl = NCAND * RSH
                    elist = ecp.tile([128, nl // 16], i16, tag="elist")
                    for r in range(8):
                        nc.sync.dma_start(
                            elist[r * 16:(r + 1) * 16, :],
                            DIDX[:].rearrange("(s p) -> p s", p=16))

                    acc = acc_l[li]
                    nc.vector.memset(acc[:], 0.0)

                    # tail refine: exact r = 2*x_m.x_j - sq_j for slots 16..23
                    r_all = ecp.tile([128, CH * NTAIL], f32, tag="rall")
                    rv = r_all[:].rearrange("p (c t) -> p c t", t=NTAIL)
                    for kk in range(NFIX, NCAND):
                        gx = ep.tile([128, CH * txw], f32, tag="gx")
                        gxv = gx[:].rearrange("p (c e) -> p c e", e=txw)
                        nc.gpsimd.dma_gather(
                            gxv, TXa, elist[:, kk * 128:(kk + 1) * 128],
                            RSH, RSH, txw, single_packet=False)
                        prod = ep.tile([128, CH * Fin], f32, tag="scr")
                        pv = prod[:].rearrange("p (c f) -> p c f", f=Fin)
                        nc.vector.tensor_tensor(pv, gxv[:, :, 0:Fin], xrow3d,
                                                op=Alu.mult)
                        dot = ep.tile([128, CH, 1], f32, tag="dot")
                        nc.vector.reduce_sum(dot[:], pv, axis=X)
                        sqg = ep.tile([128, CH * Fin], f32, tag="scr")
                        sgv = sqg[:].rearrange("p (c f) -> p c f", f=Fin)
                        nc.scalar.square(sgv, gxv[:, :, 0:Fin])
                        sqv = ep.tile([128, CH, 1], f32, tag="sqv")
                        nc.vector.reduce_sum(sqv[:], sgv, axis=X)
                        nc.vector.scalar_tensor_tensor(
                            rv[:, :, kk - NFIX:kk - NFIX + 1],
                            dot[:], 2.0, sqv[:],
                            op0=Alu.mult, op1=Alu.subtract)
                    srt = ecp.tile([128, CH * NTAIL], f32, tag="srt")
                    sv = srt[:].rearrange("p (c t) -> p c t", t=NTAIL)
                    for b in range(CH):
                        nc.vector.max(sv[:, b, :], rv[:, b, :])
                    flags = ecp.tile([128, CH * NTAIL], f32, tag="flags")
                    fv = flags[:].rearrange("p (c t) -> p c t", t=NTAIL)
                    thr_b, rv_b = bass.broadcast_tensor_aps(sv[:, :, 3:4], rv)
                    nc.vector.tensor_tensor(fv, rv_b, thr_b, op=Alu.is_ge)

                    # aggregate all 24 slots (tail slots flag-masked)
                    for kk in range(NCAND):
                        gp = ep.tile([128, CH * E], f32, tag="gp")
                        nc.gpsimd.dma_gather(
                            gp[:].rearrange("p (c e) -> p c e", e=E),
                            TPa, elist[:, kk * 128:(kk + 1) * 128],
                            RSH, RSH, E, single_packet=False)
                        tmp = ep.tile([128, CH * E], f32, tag="tmp")
                        nc.vector.tensor_add(tmp[:], gp[:], A_flat)
                        nc.vector.scalar_tensor_tensor(
                            tmp[:], tmp[:], SLOPE, tmp[:],
                            op0=Alu.mult, op1=Alu.max)
                        if kk >= NFIX:
                            tv = tmp[:].rearrange("p (c e) -> p c e", e=E)
                            fl_b, tv_b = bass.broadcast_tensor_aps(
                                fv[:, :, kk - NFIX:kk - NFIX + 1], tv)
                            nc.vector.tensor_tensor(tv_b, tv_b, fl_b, op=Alu.mult)
                        nc.vector.tensor_add(acc[:], acc[:], tmp[:])
                return acc

            # ---------------- inter-layer ----------------
            def interlayer(li, acc, Wbh, Waug):
                Snext = SCALES[li + 1]
                ag1_out = dram.tile([NC * (E + 1), RSH], f32, tag=f"ag1o{li}",
                                    name=f"ag1o{li}", addr_space="Shared")
                ag2_out = dram.tile([N, E], f32, tag=f"ag2o{li}",
                                    name=f"ag2o{li}", addr_space="Shared")
                with tc.tile_pool(name=f"il{li}", bufs=2) as ip, \
                     tc.tile_pool(name=f"psumT{li}", bufs=2, space="PSUM") as pt:
                    accv = acc[:].rearrange("p (c e) -> p c e", e=E)
                    sq2 = ip.tile([128, CH * E], f32, tag="sq2")
                    nc.scalar.square(sq2[:], acc[:])
                    sqr = ip.tile([128, CH, 1], f32, tag="sqr")
                    nc.vector.reduce_sum(
                        sqr[:], sq2[:].rearrange("p (c e) -> p c e", e=E), axis=X)
                    nc.vector.tensor_scalar(sqr[:], sqr[:], float(Snext), None,
                                            op0=Alu.mult)
                    xt = xT_loc[li + 1]
                    for b in range(CH):
                        tp = pt.tile([E, 128], f32, tag="tp")
                        nc.tensor.transpose(tp[:], accv[:, b, :], ident_sb[:])
                        nc.scalar.copy(xt[:, b * 128:(b + 1) * 128], tp[:])
                    nc.sync.dma_start(ag1_in[0:E, :], xt[:])
                    nc.sync.dma_start(
                        ag1_in[E:E + 1, :].rearrange("o (b p) -> p b o", p=128),
                        sqr[:])
                    nc.sync.dma_start(
                        ag2_in[:].rearrange("(b p) e -> p b e", p=128), accv)
                    nc.gpsimd.collective_compute(
                        "AllGather", Alu.bypass, ins=[ag1_in.opt()],
                        outs=[ag1_out.opt()], replica_groups=[list(range(NC))])
                    nc.gpsimd.collective_compute(
                        "AllGather", Alu.bypass, ins=[ag2_in.opt()],
                        outs=[ag2_out.opt()], replica_groups=[list(range(NC))])

                    xr = pp.tile([E + 2, N], f32, tag="xrbig")
                    nc.sync.dma_start(
                        xr[0:E, :].rearrange("e (r c) -> e r c", r=NC),
                        ag1_out[:].rearrange("(r e) c -> e r c", r=NC)[0:E, :, :])
                    nc.sync.dma_start(
                        xr[E:E + 1, :].rearrange("e (r c) -> e r c", r=NC),
                        ag1_out[:].rearrange("(r e) c -> e r c", r=NC)[E:E + 1, :, :])
                    nc.vector.tensor_scalar(xr[0:E, :], xr[0:E, :],
                                            float(2.0 * Snext), None, op0=Alu.mult)
                    nc.sync.dma_start(xr[E + 1:E + 2, :], BROWN.ap())
                    lh = pp.tile([E + 2, RSH], f32, tag="lhsT")
                    nc.vector.tensor_copy(lh[0:E, :], xt[:])
                    nc.vector.memset(lh[E:E + 1, :], -1.0)
                    nc.sync.dma_start(lh[E + 1:E + 2, :], PONE.ap())
                    a2 = pp.tile([128, CH * E], f32, tag="Atile")
                    psb = pp.tile([128, (N // 4 // 128) * E], f32, tag="psb")
                    with tc.tile_pool(name=f"psumA{li}", bufs=2, space="PSUM") as pa:
                        for b in range(CH):
                            aps = pa.tile([128, E], f32, tag="aps")
                            nc.tensor.matmul(aps[:], lh[0:E + 1, b * 128:(b + 1) * 128],
                                             Waug[:], start=True, stop=True)
                            nc.scalar.copy(a2[:, b * E:(b + 1) * E], aps[:])
                        for half in range(4):
                            for j4 in range(N // 4 // 512):
                                pps2 = pa.tile([128, 4 * E], f32, tag="pps2")
                                for jj in range(4):
                                    jg = half * (N // 4 // 128) + j4 * 4 + jj
                                    nc.tensor.matmul(
                                        pps2[:, jj * E:(jj + 1) * E],
                                        xr[0:E, jg * 128:(jg + 1) * 128],
                                        Wbh[:], start=True, stop=True)
                                nc.scalar.copy(
                                    psb[:, j4 * 4 * E:(j4 + 1) * 4 * E], pps2[:])
                            nc.sync.dma_start(
                                TP[half * (N // 4):(half + 1) * (N // 4), :]
                                .rearrange("(jb p) e -> p jb e", p=128),
                                psb[:].rearrange("p (jb e) -> p jb e", e=E))
                    return xr, lh, a2, ag2_out

            # ---------------- layer 1 ----------------
            xr1 = pp.tile([F1 + 2, N], f32, tag="xrbig")
            nc.sync.dma_start(xr1[:], XR1.ap())
            la1 = pp.tile([F1 + 2, RSH], f32, tag="lhsT")
            nc.sync.dma_start(la1[:], LA1.ap())
            xrow1 = pp.tile([128, CH * F1], f32, tag="xrow1")
            nc.sync.dma_start(xrow1[:], XROW1.ap())
            a1 = pp.tile([128, CH * E], f32, tag="Atile")
            nc.sync.dma_start(a1[:], A1.ap())

            do_layer(0, xr1[:], la1[:], F1,
                     xrow1[:].rearrange("p (c f) -> p c f", f=F1),
                     a1[:], a1[:].rearrange("p (c e) -> p c e", e=E),
                     TX1.ap(), TXW1, TP1.ap())

            xr2, lh2, a2, ag2o_0 = interlayer(0, acc_l[0], ws["Wb2h"], ws["W2aug"])
            do_layer(1, xr2[:], lh2[:], E,
                     acc_l[0][:].rearrange("p (c f) -> p c f", f=E),
                     a2[:], a2[:].rearrange("p (c e) -> p c e", e=E),
                     ag2o_0[:], E, TP[:])

            xr3, lh3, a3, ag2o_1 = interlayer(1, acc_l[1], ws["Wb3h"], ws["W3aug"])
            do_layer(2, xr3[:], lh3[:], E,
                     acc_l[1][:].rearrange("p (c f) -> p c f", f=E),
                     a3[:], a3[:].rearrange("p (c e) -> p c e", e=E),
                     ag2o_1[:], E, TP[:])

            # x3^T for trailing
            with tc.tile_pool(name="psumT3", bufs=2, space="PSUM") as pt3:
                accv = acc_l[2][:].rearrange("p (c e) -> p c e", e=E)
                for b in range(CH):
                    tp = pt3.tile([E, 128], f32, tag="tp3")
                    nc.tensor.transpose(tp[:], accv[:, b, :], ident_sb[:])
                    nc.scalar.copy(xT_loc[0][:, b * 128:(b + 1) * 128], tp[:])

            # ---------------- trailing MLP ----------------
            with tc.tile_pool(name="trail", bufs=2) as tr, \
                 tc.tile_pool(name="psumM", bufs=2, space="PSUM") as pm:
                wc_sb = pp.tile([E, 3 * 512], f32, tag="Atile")
                nc.sync.dma_start(
                    wc_sb[:].rearrange("f (l o) -> f l o", o=512),
                    WC.ap().rearrange("(l f) o -> f l o", f=E))
                wm1_sb = pp.tile([128, 4 * 256], f32, tag="lhsT")
                nc.sync.dma_start(
                    wm1_sb[:].rearrange("p (q o) -> p q o", o=256),
                    WM1.ap().rearrange("(q p) o -> p q o", p=128))
                wm2_sb = tr.tile([128, 2 * 128], f32, tag="wm2", bufs=1)
                nc.sync.dma_start(
                    wm2_sb[:].rearrange("p (q o) -> p q o", o=128),
                    WM2.ap().rearrange("(q p) o -> p q o", p=128))
                h1T = [pp.tile([128, RSH], f32, tag="xrbig", name="h1T0")] + [
                    tr.tile([128, RSH], f32, tag=tg, name=f"h1T_{tg}", bufs=1)
                    for tg in ("h1T1", "h1T2", "h1T3")]
                h2T = [tr.tile([128, RSH], f32, tag=tg, name=tg, bufs=1)
                       for tg in ("h2T0", "h2T1")]
                featT = pp.tile([128, RSH], f32, tag="xrow1")
                feat_sb = pp.tile([128, CH * 128], f32, tag="psb")

                xts = [xT_loc[1], xT_loc[2], xT_loc[0]]
                wm1v = wm1_sb[:].rearrange("p (q o) -> p q o", o=256)
                wm2v = wm2_sb[:].rearrange("p (q o) -> p q o", o=128)
                for b in range(CH):
                    hps = pm.tile([128, 512], f32, tag="mm")
                    for l3 in range(3):
                        nc.tensor.matmul(hps[:], xts[l3][:, b * 128:(b + 1) * 128],
                                         wc_sb[:, l3 * 512:(l3 + 1) * 512],
                                         start=(l3 == 0), stop=(l3 == 2))
                    hsb = tr.tile([128, 512], f32, tag="hsb")
                    nc.scalar.copy(hsb[:], hps[:])
                    for q in range(4):
                        tp = pm.tile([128, 128], f32, tag="tpq")
                        nc.tensor.transpose(tp[:], hsb[:, q * 128:(q + 1) * 128],
                                            ident_sb[:])
                        nc.scalar.copy(h1T[q][:, b * 128:(b + 1) * 128], tp[:])
                for b in range(CH):
                    h2ps = pm.tile([128, 256], f32, tag="mm")
                    for q in range(4):
                        nc.tensor.matmul(h2ps[:], h1T[q][:, b * 128:(b + 1) * 128],
                                         wm1v[:, q, :], start=(q == 0), stop=False)
                    nc.tensor.matmul(h2ps[:], onesn_sb[:], ws["BM1"][:],
                                     start=False, stop=True)
                    h2sb = tr.tile([128, 256], f32, tag="h2sb")
                    nc.scalar.copy(h2sb[:], h2ps[:])
                    nc.vector.scalar_tensor_tensor(
                        h2sb[:], h2sb[:], SLOPE, h2sb[:],
                        op0=Alu.mult, op1=Alu.max)
                    for q in range(2):
                        tp = pm.tile([128, 128], f32, tag="tpq")
                        nc.tensor.transpose(tp[:], h2sb[:, q * 128:(q + 1) * 128],
                                            ident_sb[:])
                        nc.scalar.copy(h2T[q][:, b * 128:(b + 1) * 128], tp[:])
                for b in range(CH):
                    fps = pm.tile([128, 128], f32, tag="mm")
                    for q in range(2):
                        nc.tensor.matmul(fps[:], h2T[q][:, b * 128:(b + 1) * 128],
                                         wm2v[:, q, :], start=(q == 0), stop=False)
                    nc.tensor.matmul(fps[:], onesn_sb[:], ws["BM2"][:],
                                     start=False, stop=True)
                    nc.scalar.copy(feat_sb[:, b * 128:(b + 1) * 128], fps[:])
                    nc.vector.scalar_tensor_tensor(
                        feat_sb[:, b * 128:(b + 1) * 128],
                        feat_sb[:, b * 128:(b + 1) * 128], SLOPE,
                        feat_sb[:, b * 128:(b + 1) * 128],
                        op0=Alu.mult, op1=Alu.max)
                    tp = pm.tile([128, 128], f32, tag="tpq")
                    nc.tensor.transpose(tp[:], feat_sb[:, b * 128:(b + 1) * 128],
                                        ident_sb[:])
                    nc.scalar.copy(featT[:, b * 128:(b + 1) * 128], tp[:])
                nc.sync.dma_start(
                    OUT_F.ap().rearrange("(b p) f -> p b f", p=128),
                    feat_sb[:].rearrange("p (b f) -> p b f", f=128))
                for b in range(CH):
                    lps = pm.tile([128, 2], f32, tag="mm")
                    nc.tensor.matmul(lps[:], featT[:, b * 128:(b + 1) * 128],
                                     ws["WCLF"][:], start=True, stop=False)
                    nc.tensor.matmul(lps[:], onesn_sb[:], ws["BCLF"][:],
                                     start=False, stop=True)
                    lsb = tr.tile([128, 2], f32, tag="lsb")
                    nc.scalar.copy(lsb[:], lps[:])
                    nc.sync.dma_start(OUT_L.ap()[b * 128:(b + 1) * 128, :], lsb[:])

    nc.compile()
    return nc


def _host_prep(inputs):
    x = np.ascontiguousarray(inputs["x"], dtype=np.float32)
    sq = (x * x).sum(1).astype(np.float32)
    Wa1, Wb1, t1 = _fold(inputs["W1"], inputs["g1"], inputs["b1"], inputs["m1"], inputs["v1"])
    Wa2, Wb2, t2 = _fold(inputs["W2"], inputs["g2"], inputs["b2"], inputs["m2"], inputs["v2"])
    Wa3, Wb3, t3 = _fold(inputs["W3"], inputs["g3"], inputs["b3"], inputs["m3"], inputs["v3"])
    S1, S2, S3 = SCALES
    P1 = (x @ Wb1).astype(np.float32)
    TX1 = np.zeros((N, TXW1), np.float32)
    TX1[:, :F1] = x

    common = {
        "XR1": np.concatenate([2.0 * S1 * x.T, S1 * sq[None, :],
                               np.full((1, N), BH, np.float32)], 0).astype(np.float32),
        "TP1": P1, "TX1": TX1,
        "IOTA": np.broadcast_to(np.arange(2 * SEG, dtype=np.float32) % SEG, (128, 2 * SEG)).copy(),
        "ONESN": -np.ones((1, 128), np.float32),
        "BROWN": np.full((1, N), BH, np.float32),
        "PONE": np.ones((1, RSH), np.float32),
        "IDENT": np.eye(128, dtype=np.float32),
        "W2aug": np.concatenate([Wa2, -t2[None, :]], 0),
        "W3aug": np.concatenate([Wa3, -t3[None, :]], 0),
        "Wb2h": (Wb2 / (2.0 * S2)).astype(np.float32),
        "Wb3h": (Wb3 / (2.0 * S3)).astype(np.float32),
        "WC": np.ascontiguousarray(inputs["Wc"], np.float32),
    }
    s4 = (inputs["g4"] / np.sqrt(inputs["v4"] + EPS)).astype(np.float32)
    t4 = (inputs["b4"] - inputs["m4"] * s4).astype(np.float32)
    common["WM1"] = (inputs["Wm1"] * s4[None, :]).astype(np.float32)
    common["BM1"] = -(inputs["bm1"] * s4 + t4)[None, :].astype(np.float32)
    s5 = (inputs["g5"] / np.sqrt(inputs["v5"] + EPS)).astype(np.float32)
    t5 = (inputs["b5"] - inputs["m5"] * s5).astype(np.float32)
    common["WM2"] = (inputs["Wm2"] * s5[None, :]).astype(np.float32)
    common["BM2"] = -(inputs["bm2"] * s5 + t5)[None, :].astype(np.float32)
    common["WCLF"] = np.ascontiguousarray(inputs["Wclf"], np.float32)
    common["BCLF"] = -np.ascontiguousarray(inputs["bclf"], np.float32)[None, :]

    in_maps = []
    for c in range(NC):
        xs = x[c * RSH:(c + 1) * RSH]
        m = dict(common)
        m["LA1"] = np.concatenate([xs.T, -np.ones((1, RSH), np.float32),
                                   np.ones((1, RSH), np.float32)], 0).astype(np.float32)
        m["XROW1"] = np.ascontiguousarray(
            xs.reshape(CH, 128, F1).transpose(1, 0, 2).reshape(128, CH * F1))
        A1v = (xs @ Wa1 + t1[None, :]).astype(np.float32)
        m["A1"] = np.ascontiguousarray(
            A1v.reshape(CH, 128, E).transpose(1, 0, 2).reshape(128, CH * E))
        in_maps.append(m)
    return in_maps


def kernel(**inputs):
    import concourse.bacc as bacc
    from concourse.bass_utils import run_bass_kernel_spmd

    if "nc" not in _CACHE:
        ncb = bacc.Bacc("TRN2", target_bir_lowering=False, debug=False,
                        num_devices=NC)
        _CACHE["nc"] = _build(ncb)
    ncb = _CACHE["nc"]
    in_maps = _host_prep(inputs)
    res = run_bass_kernel_spmd(ncb, in_maps, core_ids=list(range(NC)))
    logits = np.concatenate([res.results[c]["OUT_L"] for c in range(NC)], 0)
    feat = np.concatenate([res.results[c]["OUT_F"] for c in range(NC)], 0)
    return logits, feat
